# revision 35
# baseline (speedup 1.0000x reference)
"""Trainium2 Bass kernel for nn_LilletLayer (gnn_message_passing).

Math (per molecule b, per head h):
  xc = W_map @ x   (K=6 coarse particles, 3d coords)
  delta over K*K (k1,k2) pairs -> ExpNormalSmearing -> basis (36, 50, 3)
  att[a,c,n] = sum_x basis[a,n,x]*basis[c,n,x]
  out = silu(att @ W1 + b1) @ W2 + b2

Exact algebraic folds (validated vs the reference in fp32):
 1. basis[a,n,x] = deltam[x,a]*g[a,n] is separable, so
      att[a,c,n] = D2[a,c]*g[a,n]*g[c,n],  D2 = deltam^T deltam.
 2. Mirror/diagonal pair symmetry collapses the (36x36) pair-pair
    contraction onto the 120 upper-triangular pair-pairs of the 15
    canonical (k1<k2) pairs: W1 folded host-side. 6000 rows per head.
 3. The cutoff/denominator factor m3[a] = (cos(pi*d_a/5)+1)/(d_a+1e-6)^2
    folds into the pairwise gram: d2fm[a,c] = D2[a,c]*m3[a]*m3[c].
    cos(pi*min(d,5)/5)+1 is evaluated as a degree-5 polynomial in
    w = min(d^2,25)/25 on the DVE (cos(pi*sqrt(w)) is analytic in w),
    keeping Sin off the ACT engine so its PWP tables never evict Exp.

Sharding (v2): tensor-parallel over heads. Core r computes head r's
partial pre-activation h1 contribution for ALL 128 molecules:
 - per-core W1 slice is 1/8 of the folded weights (1.54 MB vs 12.3 MB),
   stored partition-major in DRAM so each partition reads one contiguous
   multi-KB run (the old layout's 256B packets capped HBM at ~115 GB/s
   and made the W1 load the 67us critical path of the 78us kernel).
 - the DVE/ACT preamble cost is unchanged (128 partition rows either way)
   and the 47 chunk matmuls are now full 128-wide instead of 8x16-wide.
 - the cross-head sum + silu + W2 + b2 happen on the host during the
   gather/unshard step (an on-device ReduceScatter was measured at
   20-45us fixed latency on this runtime - slower than the whole rest
   of the kernel; the host epilogue is 0.13 MFLOP, far below the xc
   einsum already computed host-side).

Hardware notes baked in (from per-instruction trace analysis):
 - The DVE runs 2-byte ops at 2x ONLY if no source has an
   innermost-stride-0 (element-repeat) access. Outer broadcasts are
   fine. So d2fm is stored pair-doubled ([15,15,2], value duplicated)
   and the att multiply reads it as [[2,cc],[0,25],[1,2]] - packed
   innermost - while the g2 multiply uses the a-row as an outer
   broadcast. Both att-construction ops then run at 2x.
 - ACT PWP tables evict each other; order is Sqrt -> Exp -> Square ->
   Exp (square lives in every set); no Sigmoid needed on device.
 - att rows live packed [b, 6000]; the tail chunk contracts only its
   112 valid rows, so no pad initialization is needed anywhere.
 - PSUM->SBUF copies of the transposed att chunks ride the otherwise
   idle Scalar(ACT) engine so the DVE stays dedicated to att rows.
"""

import math

import numpy as np

import concourse.bacc as bacc
import concourse.bass as bass
import concourse.mybir as mybir
import concourse.tile as tile
from concourse.bass_utils import run_bass_kernel_spmd
from concourse.masks import make_identity

B, N, H, K, R = 128, 512, 8, 6, 50
CUT = 5.0
P15 = K * (K - 1) // 2        # 15 canonical (k1<k2) pairs
FTOT = P15 * (P15 + 1) // 2 * R  # 6000 true contraction rows per head
SPANS = [(P15 - a) * R for a in range(P15)]
OFFS = np.concatenate([[0], np.cumsum(SPANS)]).astype(int)
# quad-merged att layout: a-blocks grouped (4,4,4,3), each group padded to
# its first block's width so one DVE op builds the whole group. Pad rows
# multiply memset-zero g rows and map to zero rows of the folded W1.
QUADS = [(0, 4, 15), (4, 4, 11), (8, 4, 7), (12, 3, 3)]  # (a0, w, cc)
QOFF = [0]
for _a0, _w, _cc in QUADS:
    QOFF.append(QOFF[-1] + _w * _cc * R)
FTOTQ = QOFF[-1]              # 7050 padded rows
NCH = (FTOTQ + 127) // 128    # 56 chunks
FPAD = NCH * 128              # 7168
NTAIL = FTOTQ - (NCH - 1) * 128  # 10 valid rows in the last chunk
GRP = 6                       # transpose/matmul group size (chunks)
NWARM = 22                    # PE clock-ramp warmup transposes
HID = 128
F32 = mybir.dt.float32
BF16 = mybir.dt.bfloat16
AF = mybir.ActivationFunctionType
ALU = mybir.AluOpType

# cos(pi*sqrt(w)) on [0,1], max err 8e-7; coef 0 carries the cutoff's +1
_PC = [0.9999991998413434, -4.934744543965318, 4.058036739995789,
       -1.3323569316702395, 0.2296364873552529, -0.020571708405640265]


def _bcast(ap, axis, count):
    """Insert a stride-0 (broadcast) free dim at free-axis position `axis`."""
    dims = [list(d) for d in ap.ap]
    dims.insert(axis + 1, [0, count])  # +1: dims[0] is the partition dim
    return bass.AP(tensor=ap.tensor, offset=ap.offset, ap=dims)


def _with_dims(ap, dims):
    """Replace the free dims of `ap` with explicit [step, count] pairs."""
    return bass.AP(
        tensor=ap.tensor, offset=ap.offset, ap=[list(ap.ap[0])] + [list(d) for d in dims]
    )


def _view(ap, extra_off, dims):
    """Free-dim view at `ap.offset + extra_off` with explicit dims."""
    return bass.AP(
        tensor=ap.tensor, offset=ap.offset + extra_off,
        ap=[list(ap.ap[0])] + [list(d) for d in dims],
    )


def build_program(n_cores=8):
    nc = bacc.Bacc(
        "TRN2",
        target_bir_lowering=False,
        debug=False,
        enable_asserts=False,
        num_devices=n_cores,
    )

    xcin = nc.dram_tensor("xcin", [B, 3, K], F32, kind="ExternalInput").ap()
    # this core's head-slice of folded W1, partition-major:
    # w1s[p, j, hid] so each partition's slab read is contiguous
    w1s = nc.dram_tensor("w1s", [128, NCH * HID], BF16, kind="ExternalInput").ap()
    mrep = nc.dram_tensor("mrep", [B, R], F32, kind="ExternalInput").ap()
    nbs = nc.dram_tensor("nbs", [B, 1], F32, kind="ExternalInput").ap()
    outd = nc.dram_tensor("out", [HID, B], F32, kind="ExternalOutput").ap()

    with tile.TileContext(nc) as tc:
        with (
            tc.tile_pool(name="singles", bufs=1) as singles,
            tc.tile_pool(name="g2p", bufs=2) as g2p,
            tc.tile_pool(name="attTp", bufs=4) as attT_pool,
            tc.tile_pool(name="ps_acc", bufs=1, space="PSUM") as ps_acc_pool,
            tc.tile_pool(name="ps_t", bufs=5, space="PSUM") as ps_t_pool,
            tc.tile_pool(name="ps_warm", bufs=1, space="PSUM") as ps_warm_pool,
        ):
            # ---------------- input DMAs ------------------------------------
            xc_sb = singles.tile([128, 3, K], F32)
            nc.sync.dma_start(out=xc_sb, in_=xcin)
            mrep_sb = singles.tile([128, R], F32)
            nc.sync.dma_start(out=mrep_sb, in_=mrep)
            nbs_sb = singles.tile([128, 1], F32)
            nc.sync.dma_start(out=nbs_sb, in_=nbs)
            # this head's folded W1 resident in SBUF: [128, NCH, HID] bf16.
            # Each partition's DRAM bytes are contiguous per slab, so the
            # DMA runs multi-KB descriptors at full HBM rate. Split across
            # the sync (HWDGE) and gpsimd (SWDGE) queues.
            w1all = singles.tile([128, NCH, HID], BF16)

            def _w1_slab(eng, j0, j1):
                return eng.dma_start(
                    out=w1all[:, j0:j1],
                    in_=bass.AP(
                        tensor=w1s.tensor, offset=j0 * HID,
                        ap=[[NCH * HID, 128], [1, (j1 - j0) * HID]],
                    ),
                )

            # all W1 on the sync HWDGE queue: the GPSIMD queue must stay
            # free for the gram/att ops below (its SWDGE drain would
            # otherwise block them until the transfers land)
            _w1_slab(nc.sync, 0, 14)
            _w1_slab(nc.sync, 14, 28)
            _w1_slab(nc.sync, 28, 42)
            _w1_slab(nc.sync, 42, NCH)

            # ---------------- constants -------------------------------------
            ident = singles.tile([128, 128], BF16)
            make_identity(nc, ident)
            warm = singles.tile([128, 1], F32)
            warmo = singles.tile([128, 1], F32)
            warmsrc = singles.tile([128, 128], BF16)
            nc.gpsimd.memset(warmsrc, 0.0)
            nc.gpsimd.memset(warm, 1.0)
            # ACT: pull the Sqrt table load off the dnorm critical edge
            nc.scalar.activation(warmo, warm[:, 0:1], AF.Sqrt)

            # ------------- delta over the 15 canonical (k1<k2) pairs --------
            delta_sb = singles.tile([128, 3, P15], F32)
            off = 0
            for q1 in range(K - 1):
                cnt = K - 1 - q1
                nc.vector.tensor_sub(
                    delta_sb[:, :, off:off + cnt],
                    _bcast(xc_sb[:, :, q1], 1, cnt),
                    xc_sb[:, :, q1 + 1:],
                )
                off += cnt

            # d2[b, a] = sum_d delta^2 ; dnorm = sqrt(d2)
            d2sq_sb = singles.tile([128, P15, 3], F32)
            nc.vector.tensor_mul(
                d2sq_sb,
                _with_dims(delta_sb[:], [[1, P15], [P15, 3]]),
                _with_dims(delta_sb[:], [[1, P15], [P15, 3]]),
            )
            d2_sb = singles.tile([128, P15], F32)
            nc.vector.tensor_reduce(
                d2_sb, d2sq_sb, axis=mybir.AxisListType.X, op=ALU.add
            )
            dnorm_sb = singles.tile([128, P15], F32)
            nc.scalar.activation(dnorm_sb, d2_sb, AF.Sqrt)

            # -------- pairwise gram (GPSIMD): d2f = delta.delta -------------
            # runs on the otherwise idle GPSIMD so the DVE can spend this
            # window on the cutoff polynomial instead
            q0 = singles.tile([128, P15, P15], F32)
            nc.gpsimd.tensor_mul(
                q0,
                _with_dims(delta_sb[:, 0], [[1, P15], [0, P15]]),
                _with_dims(delta_sb[:, 0], [[0, P15], [1, P15]]),
            )
            q1t = singles.tile([128, P15, P15], F32)
            nc.gpsimd.tensor_mul(
                q1t,
                _with_dims(delta_sb[:, 1], [[1, P15], [0, P15]]),
                _with_dims(delta_sb[:, 1], [[0, P15], [1, P15]]),
            )
            q01 = singles.tile([128, P15, P15], F32)
            nc.gpsimd.tensor_add(q01, q0, q1t)
            q2 = singles.tile([128, P15, P15], F32)
            nc.gpsimd.tensor_mul(
                q2,
                _with_dims(delta_sb[:, 2], [[1, P15], [0, P15]]),
                _with_dims(delta_sb[:, 2], [[0, P15], [1, P15]]),
            )
            d2f_sb = singles.tile([128, P15, P15], F32)
            nc.gpsimd.tensor_add(d2f_sb, q01, q2)

            # cutoff+1 = cos(pi*sqrt(w))+1, w = min(d2,25)/25: poly on DVE
            w_sb = singles.tile([128, P15], F32)
            nc.vector.tensor_scalar(
                w_sb, d2_sb, CUT * CUT, 1.0 / (CUT * CUT), op0=ALU.min, op1=ALU.mult
            )
            wp = [None, w_sb]
            for k in range(2, 6):
                t_ = singles.tile([128, P15], F32, tag=f"wpow{k}")
                nc.vector.tensor_mul(t_, wp[k // 2], wp[(k + 1) // 2])
                wp.append(t_)
            acc = singles.tile([128, P15], F32)
            nc.vector.tensor_scalar(
                acc, w_sb, _PC[1], _PC[0] + 1.0, op0=ALU.mult, op1=ALU.add
            )
            for k in range(2, 6):
                acc2 = singles.tile([128, P15], F32, tag=f"pacc{k}")
                nc.vector.scalar_tensor_tensor(
                    acc2, in0=wp[k], scalar=_PC[k], in1=acc,
                    op0=ALU.mult, op1=ALU.add,
                )
                acc = acc2
            # ---------------- smearing g[b, a, r] (bf16, via ACT) -----------
            # ACT chain Sqrt -> Exp -> Square -> Exp: one Exp table load.
            # t is issued (and dep-pinned) BEFORE the m3 chain so the DVE
            # feeds the ACT Square->Exp(g) chain as early as possible; the
            # gram products then fill the DVE while ACT produces g.
            e_sb = singles.tile([128, P15], F32)
            nc.scalar.activation(e_sb, dnorm_sb, AF.Exp, scale=-1.0)
            t_sb = singles.tile([128, P15, R], F32)
            t_op = nc.vector.tensor_sub(
                t_sb, _bcast(e_sb[:], 1, R), _bcast(mrep_sb[:], 0, P15)
            )
            # warmup source touch: gates the PE ramp chain to start with `t`
            nc.vector.tensor_copy(warmsrc[:, 0:1], t_sb[:, 0, 0:1])
            # (chunked Square/Exp was tried and reverted: ~330ns fixed cost
            # per ACT op makes 6 chunked ops 1.3us slower than these 2)
            tsq_sb = singles.tile([128, P15, R], F32)
            nc.scalar.activation(tsq_sb, t_sb, AF.Square)
            # g padded with 3 zero rows: the quad-merged att ops read up to
            # row a0+w-1+cc-1 = 17; the pad contributes zeros that land on
            # zero rows of the folded W1
            g_sb = singles.tile([128, (P15 + 3) * R], BF16)
            nc.gpsimd.memset(g_sb[:, P15 * R:], 0.0)
            # betas is uniform across R, so -beta folds into the Exp scale
            nc.scalar.activation(
                _with_dims(g_sb[:], [[1, P15 * R]]), tsq_sb, AF.Exp,
                scale=nbs_sb[:, 0:1],
            )

            # inv = 1/d2 ; m3 = (cutoff+1)*inv
            # (the reference's +1e-6 sqrt regularizer only matters at d ~ 0,
            # which the canonical k1<k2 pairs never hit for real inputs)
            inv_sb = singles.tile([128, P15], F32)
            inv_op = nc.vector.reciprocal(inv_sb, d2_sb)
            bass._add_dep_helper(
                inv_op.ins, t_op.ins, sync=False, reason="t feeds ACT first"
            )
            m3_sb = singles.tile([128, P15], F32)
            nc.vector.tensor_mul(m3_sb, acc, inv_sb)
            m3o_sb = singles.tile([128, P15, P15], F32)
            nc.vector.tensor_mul(
                m3o_sb,
                _with_dims(m3_sb[:], [[1, P15], [0, P15]]),
                _with_dims(m3_sb[:], [[0, P15], [1, P15]]),
            )
            # pair-doubled d2fm, flat [a*32 + c*2 + p] with a zeroed tail pad
            # (the quad rearrange reads up to flat index 453): the att
            # multiply then reads packed innermost pairs instead of an
            # element-repeat broadcast
            d2fme_sb = singles.tile([128, 460], BF16)
            nc.gpsimd.memset(d2fme_sb[:, 450:], 0.0)
            nc.vector.tensor_mul(
                _with_dims(d2fme_sb[:], [[2, P15 * P15], [1, 2]]),
                _with_dims(d2f_sb[:], [[1, P15 * P15], [0, 2]]),
                _with_dims(m3o_sb[:], [[1, P15 * P15], [0, 2]]),
            )

            # PE clock ramp: back-to-back transposes starting at `t`, ending
            # as the first real transpose becomes ready
            ps_warm = ps_warm_pool.tile([128, 128], BF16, tag="warm")
            for _ in range(NWARM):
                nc.tensor.transpose(ps_warm, warmsrc, ident)

            # ---------------- att rows, quad-merged [b, 7050] bf16 ----------
            # three DVE ops per GROUP of 3-4 a-blocks (vs 2 per block):
            # rearrange the needed d2fm pairs, one big gg product, one big
            # d2fm multiply. Cuts the 30-op stream to 14 ops; ~200ns fixed
            # cost per DVE op made the op count the dominant term.
            # (GPSIMD att assist was tried and reverted: the two engines
            # contend for SBUF bandwidth, slowing DVE att ops ~1.5x.)
            att_sb = singles.tile([128, FPAD], BF16)
            for q, (a0, w, cc) in enumerate(QUADS):
                span = w * cc * R
                base = QOFF[q]
                d2fq_t = g2p.tile([128, 120], BF16, tag="d2fq")
                nc.vector.tensor_copy(
                    _with_dims(d2fq_t[:], [[cc * 2, w], [2, cc], [1, 2]]),
                    _view(d2fme_sb[:], a0 * 32, [[32, w], [2, cc], [1, 2]]),
                )
                g2_t = g2p.tile([128, 3000], BF16, tag="g2")
                nc.vector.tensor_mul(
                    _with_dims(g2_t[:], [[cc * R, w], [R, cc], [1, R]]),
                    _view(g_sb[:], a0 * R, [[R, w], [0, cc], [1, R]]),
                    _view(g_sb[:], a0 * R, [[R, w], [R, cc], [1, R]]),
                )
                nc.vector.tensor_mul(
                    _with_dims(att_sb[:, base:base + span],
                               [[R, w * cc], [2, R // 2], [1, 2]]),
                    _with_dims(g2_t[:], [[R, w * cc], [2, R // 2], [1, 2]]),
                    _with_dims(d2fq_t[:], [[2, w * cc], [0, R // 2], [1, 2]]),
                )

            # ---- PE transpose + contraction, groups of GRP chunks ----------
            # one full-width matmul per chunk; the 47 chunks accumulate this
            # head's h1 partial for all 128 molecules in PSUM. The ACT
            # engine is nearly saturated by the PSUM->SBUF copies, so two
            # mid-stream groups ride the GPSIMD and the 42-45 group rides
            # the by-then-idle DVE; the tail group is the lone chunk 46.
            ps_acc = ps_acc_pool.tile([HID, B], F32)
            bounds = list(range(0, 48, GRP)) + [48, 52, 55, NCH]
            ngrp = len(bounds) - 1

            def emit_transposes(gi):
                lo, hi = bounds[gi], bounds[gi + 1]
                pst = ps_t_pool.tile([128, GRP, B], BF16, tag="pst")
                for j in range(lo, hi):
                    nc.tensor.transpose(
                        pst[:, j - lo], att_sb[:, j * 128:(j + 1) * 128], ident
                    )
                attT = attT_pool.tile([128, GRP, B], BF16, tag="attT")
                if lo == 52:
                    nc.vector.tensor_copy(attT[:, :hi - lo], pst[:, :hi - lo])
                else:
                    # bitcast the bf16 pairs to f32 so the per-element ACT
                    # copy moves half as many elements
                    nc.scalar.activation(
                        attT[:, :hi - lo].bitcast(F32),
                        pst[:, :hi - lo].bitcast(F32),
                        AF.Copy,
                    )
                return attT

            def emit_matmuls(gi, attT):
                lo, hi = bounds[gi], bounds[gi + 1]
                for j in range(lo, hi):
                    kr = NTAIL if j == NCH - 1 else 128
                    nc.tensor.matmul(
                        ps_acc,
                        lhsT=w1all[:kr, j],
                        rhs=attT[:kr, j - lo],
                        start=(j == 0),
                        stop=(j == NCH - 1),
                    )

            # mid-stream groups run transpose->copy->matmul per group; the
            # last three groups' transposes are hoisted ahead of the
            # (copy-blocked) matmuls so the PE queue order cannot serialize
            # the tail into three full group-latencies
            for gi in range(ngrp - 3):
                attT = emit_transposes(gi)
                emit_matmuls(gi, attT)
            tails = [emit_transposes(gi) for gi in range(ngrp - 3, ngrp)]
            for k, gi in enumerate(range(ngrp - 3, ngrp)):
                emit_matmuls(gi, tails[k])

            # ---------------- partial h1 out (host finishes the head sum) ---
            hb_sb = singles.tile([HID, B], F32)
            nc.vector.tensor_copy(hb_sb, ps_acc)
            nc.sync.dma_start(out=outd, in_=hb_sb)

    nc.compile()
    return nc


def host_prep(x, W_map, means, betas, W1, b1, W2, b2):
    """Build the 8 per-core input maps (numpy)."""
    import ml_dtypes

    x = np.ascontiguousarray(np.asarray(x, np.float32))
    W_map = np.asarray(W_map, np.float32)
    means = np.asarray(means, np.float32)
    betas = np.asarray(betas, np.float32)
    W1 = np.asarray(W1, np.float32)

    # coarse-grained coords per head, computed host-side (trivial FLOPs)
    xc_h = np.einsum('hkn,bnd->hbdk', W_map, x).astype(np.float32)

    # Fold W1 (H, 36, 36, R, HID) onto the 15 canonical pairs with mirror
    # signs, then onto the 120 upper-triangular pair-pairs (a-major order).
    P36 = K * K
    canon = [(i, j) for i in range(K) for j in range(i + 1, K)]
    a_of = np.array([i * K + j for (i, j) in canon])
    abar = np.array([j * K + i for (i, j) in canon])
    W1r = W1.reshape(H, P36, P36, R, HID)
    W1q = (
        W1r[:, a_of[:, None], a_of[None, :]]
        - W1r[:, a_of[:, None], abar[None, :]]
        - W1r[:, abar[:, None], a_of[None, :]]
        + W1r[:, abar[:, None], abar[None, :]]
    )  # (H, 15, 15, R, HID)
    tri_a, tri_c = np.triu_indices(P15)
    W1t = W1q[:, tri_a, tri_c] + np.where(
        (tri_a != tri_c)[None, :, None, None], W1q[:, tri_c, tri_a], 0.0
    )  # (H, 120, R, HID)
    # x0.25: device gram factors are 2x ref (cutoff computed as cos+1)
    W1flat = (W1t * 0.25).reshape(H, FTOT, HID)
    # scatter onto the quad-padded device column order: padded column
    # (q, i, c, n) <- canonical row (a0+i, a0+i+c, n), or zero if the pair
    # index exceeds the triangle (those att columns are zeros too)
    W1s_dev = np.zeros((H, FPAD, HID), np.float32)
    for q, (a0, w, cc) in enumerate(QUADS):
        for i in range(w):
            a = a0 + i
            ccv = P15 - a  # valid c count for this block
            src0 = int(OFFS[a])
            dst0 = QOFF[q] + i * cc * R
            W1s_dev[:, dst0:dst0 + ccv * R] = W1flat[:, src0:src0 + ccv * R]
    # per-head partition-major layout [p, j, hid]: contiguous per partition
    w1pm = np.ascontiguousarray(
        W1s_dev.reshape(H, NCH, 128, HID).transpose(0, 2, 1, 3)
    ).astype(ml_dtypes.bfloat16)  # (H, 128, NCH, HID)

    mrep = np.ascontiguousarray(np.broadcast_to(means, (B, R)), np.float32)
    assert np.all(betas == betas[0]), "kernel folds the uniform beta into Exp"
    nbs = np.full((B, 1), -float(betas[0]), np.float32)

    in_maps = []
    for r in range(H):
        in_maps.append(
            dict(
                xcin=np.ascontiguousarray(xc_h[r]),  # (128 mol, 3, K)
                w1s=np.ascontiguousarray(w1pm[r].reshape(128, NCH * HID)),
                mrep=mrep,
                nbs=nbs,
            )
        )
    return in_maps


_NC_CACHE = {}


def get_program():
    if "nc" not in _NC_CACHE:
        _NC_CACHE["nc"] = build_program()
    return _NC_CACHE["nc"]


def kernel(x, W_map, means, betas, W1, b1, W2, b2, _debug=False, _trace=False):
    in_maps = host_prep(x, W_map, means, betas, W1, b1, W2, b2)
    nc = get_program()
    res = run_bass_kernel_spmd(nc, in_maps, list(range(H)), trace=_trace)
    # gather/unshard: sum the per-head h1 partials, then the (tiny) tail
    # of the net: silu(h1 + b1) @ W2 + b2  -> (B, 1)
    hsum = np.zeros((HID, B), np.float32)
    for r in range(H):
        hsum += np.asarray(res.results[r]["out"], np.float32)
    b1 = np.asarray(b1, np.float32).reshape(1, HID)
    W2 = np.asarray(W2, np.float32).reshape(HID, 1)
    b2 = np.asarray(b2, np.float32).reshape(1)
    h1 = hsum.T + b1  # (B, HID)
    s = h1 * (1.0 / (1.0 + np.exp(-h1)))
    out = (s @ W2 + b2).astype(np.float32)
    if _debug or _trace:
        kernel.last_results = res
    return out
